# revision 1
# baseline (speedup 1.0000x reference)
"""GCN encoder (3x gcn_conv) on 8 Trainium2 NeuronCores.

Graph-parallel by destination node:
- Nodes are sharded 6250/core; each core owns the edges whose destination
  (col) falls in its shard, grouped into destination blocks of 128 nodes.
- The layer-1 node table h1 = x @ W1 is built (replicated) on every core.
- Per 128-edge chunk: an indirect DMA gathers the 128 source rows h[row];
  the edge-attr linear runs on the tensor engine (K=8, bias folded in as a
  ones-row); relu on ACT; scatter-add is a one-hot matmul into a PSUM
  accumulator per destination block (S[e,dst] = (iota==colrel)*dinv_row,
  built on DVE), applying dinv[row]. dinv[col] and the self-loop term are
  applied per destination block.
- Layers mu/logstd share edges and gathers: their node tables are
  concatenated into one 128-wide table T2 = [h@Wmu | h@Wls], which is
  AllGathered across the cores between the two edge passes.
"""
import numpy as np

N_NODES = 50000
N_CORES = 8
SHARD = N_NODES // N_CORES          # 6250
P = 128
NBLK = (SHARD + P - 1) // P         # 49 destination blocks / core
IN_F = 128
HID = 128
OUT_F = 64
TBLK = (N_NODES + P - 1) // P       # 391 table-build chunks


def _host_prep(x, edge_index, edge_attr,
               W1, b1, We1, be1, root1,
               Wmu, bmu, Wemu, bemu, rootmu,
               Wls, bls, Wels, bels, rootls):
    x = np.asarray(x, np.float32)
    row = np.asarray(edge_index[0], np.int64)
    col = np.asarray(edge_index[1], np.int64)
    ea = np.asarray(edge_attr, np.float32)
    E = row.shape[0]

    deg = (np.bincount(row, minlength=N_NODES) + 1.0).astype(np.float32)
    dinv = deg ** -0.5
    rdeg = (1.0 / deg).astype(np.float32)

    core_of = col // SHARD
    blk_of = (col - core_of * SHARD) // P

    # uniform chunks-per-block across cores (SPMD: one program for all)
    counts = np.zeros((N_CORES, NBLK), np.int64)
    for c in range(N_CORES):
        m = core_of == c
        counts[c] = np.bincount(blk_of[m], minlength=NBLK)
    n_chunks = np.maximum(1, (counts.max(axis=0) + P - 1) // P).astype(int)
    NCH = int(n_chunks.sum())

    offs = np.zeros((N_CORES, P, NCH), np.int32)
    colrel = np.full((N_CORES, P, NCH), -1.0, np.float32)
    srw = np.zeros((N_CORES, P, NCH), np.float32)
    at = np.zeros((N_CORES, 8, NCH * P), np.float32)
    chunk_base = np.concatenate([[0], np.cumsum(n_chunks)])[:-1]

    order = np.lexsort((blk_of, core_of))
    row_s, col_s = row[order], col[order]
    core_s, blk_s = core_of[order], blk_of[order]
    ea_s = ea[order]
    seg_cnt = np.zeros(N_CORES * NBLK + 1, np.int64)
    np.add.at(seg_cnt, core_s * NBLK + blk_s + 1, 1)
    seg_start = np.cumsum(seg_cnt)
    pos_in_seg = np.arange(E) - seg_start[core_s * NBLK + blk_s]

    chunk_idx = chunk_base[blk_s] + pos_in_seg // P
    part_idx = pos_in_seg % P

    offs[core_s, part_idx, chunk_idx] = row_s.astype(np.int32)
    colrel[core_s, part_idx, chunk_idx] = (col_s - core_s * SHARD - blk_s * P).astype(np.float32)
    srw[core_s, part_idx, chunk_idx] = dinv[row_s]
    flat = chunk_idx * P + part_idx
    for j in range(7):
        at[core_s, j, flat] = ea_s[:, j]
    at[core_s, 7, flat] = 1.0

    dinvcol = np.zeros((N_CORES, P, NBLK), np.float32)
    rdegc = np.zeros((N_CORES, P, NBLK), np.float32)
    selfoff = np.zeros((N_CORES, P, NBLK), np.int32)
    for c in range(N_CORES):
        ids = c * SHARD + np.arange(SHARD)
        b = np.arange(SHARD) // P
        p = np.arange(SHARD) % P
        dinvcol[c, p, b] = dinv[ids]
        rdegc[c, p, b] = rdeg[ids]
        selfoff[c, p, b] = ids

    W1 = np.asarray(W1, np.float32)
    we1 = np.concatenate([np.asarray(We1, np.float32),
                          (np.asarray(be1) + np.asarray(b1))[None, :]], 0).astype(np.float32)
    bias1 = np.tile((np.asarray(b1) + np.asarray(root1))[None, :], (P, 1)).astype(np.float32)
    wcat = np.concatenate([np.asarray(Wmu), np.asarray(Wls)], 1).astype(np.float32)
    we2 = np.concatenate([
        np.concatenate([np.asarray(Wemu), np.asarray(Wels)], 1),
        np.concatenate([np.asarray(bemu) + np.asarray(bmu),
                        np.asarray(bels) + np.asarray(bls)])[None, :]], 0).astype(np.float32)
    bias2 = np.tile(np.concatenate([np.asarray(bmu) + np.asarray(rootmu),
                                    np.asarray(bls) + np.asarray(rootls)])[None, :],
                    (P, 1)).astype(np.float32)
    iota = np.tile(np.arange(P, dtype=np.float32)[None, :], (P, 1))
    ident = np.eye(P, dtype=np.float32)
    xrows = np.ascontiguousarray(x)

    shared = dict(xrows=xrows, W1=W1, we1=we1, bias1=bias1, wcat=wcat, we2=we2,
                  bias2=bias2, iota=iota, ident=ident)
    # layer-2 gather offsets against the split hfull layout:
    # rows [0, 8*HALF) = ranks' first halves; rows [8*HALF, N) = second halves
    HALF = SHARD // 2
    oc = offs // SHARD          # owning core of each source row
    orr = offs - oc * SHARD     # local row within that core's shard
    offs2 = np.where(orr < HALF, oc * HALF + orr,
                     N_CORES * HALF + oc * (SHARD - HALF) + (orr - HALF)).astype(np.int32)
    per_core = []
    for c in range(N_CORES):
        d = dict(offs=offs[c], offs2=offs2[c], colrel=colrel[c], srw=srw[c],
                 at=at[c], dinvcol=dinvcol[c], rdegc=rdegc[c],
                 xself=np.ascontiguousarray(x[c * SHARD:(c + 1) * SHARD]))
        d.update(shared)
        per_core.append(d)
    return per_core, n_chunks, NCH


HALF = SHARD // 2


def _build_nc(n_chunks, NCH, phases=("tab","self","l1","ag","l2"), max_sched=None, ablate=()):
    from concourse import bass, bacc, mybir
    import concourse.tile as tile

    f32 = mybir.dt.float32
    i32 = mybir.dt.int32
    Relu = mybir.ActivationFunctionType.Relu
    Alu = mybir.AluOpType
    nc = bacc.Bacc(None, num_devices=N_CORES)

    xrows_d = nc.declare_dram_parameter("xrows", [N_NODES, IN_F], f32, isOutput=False)
    W1_d = nc.declare_dram_parameter("W1", [IN_F, HID], f32, isOutput=False)
    we1_d = nc.declare_dram_parameter("we1", [8, HID], f32, isOutput=False)
    bias1_d = nc.declare_dram_parameter("bias1", [P, HID], f32, isOutput=False)
    wcat_d = nc.declare_dram_parameter("wcat", [HID, P], f32, isOutput=False)
    we2_d = nc.declare_dram_parameter("we2", [8, P], f32, isOutput=False)
    bias2_d = nc.declare_dram_parameter("bias2", [P, P], f32, isOutput=False)
    iota_d = nc.declare_dram_parameter("iota", [P, P], f32, isOutput=False)
    ident_d = nc.declare_dram_parameter("ident", [P, P], f32, isOutput=False)
    offs_d = nc.declare_dram_parameter("offs", [P, NCH], i32, isOutput=False)
    offs2_d = nc.declare_dram_parameter("offs2", [P, NCH], i32, isOutput=False)
    colrel_d = nc.declare_dram_parameter("colrel", [P, NCH], f32, isOutput=False)
    srw_d = nc.declare_dram_parameter("srw", [P, NCH], f32, isOutput=False)
    at_d = nc.declare_dram_parameter("at", [8, NCH * P], f32, isOutput=False)
    dinvcol_d = nc.declare_dram_parameter("dinvcol", [P, NBLK], f32, isOutput=False)
    rdegc_d = nc.declare_dram_parameter("rdegc", [P, NBLK], f32, isOutput=False)
    xself_d = nc.declare_dram_parameter("xself", [SHARD, IN_F], f32, isOutput=False)
    out_d = nc.declare_dram_parameter("out", [SHARD, P], f32, isOutput=True)

    hshardA = nc.dram_tensor("hshardA", [HALF, HID], f32)
    hshardB = nc.dram_tensor("hshardB", [SHARD - HALF, HID], f32)
    hfull = nc.dram_tensor("hfull", [N_NODES, HID], f32, addr_space="Shared")

    SUP = 4      # chunks per elementwise batch
    ATSUP = 64   # chunks per edge-attr stream tile

    sched = []
    for b, nk in enumerate(n_chunks):
        for k in range(nk):
            sched.append((b, k, int(nk)))
    if max_sched is not None:
        # truncate to whole blocks
        sched = [t for t in sched if t[0] < max_sched]
        phases = tuple(phases)

    with tile.TileContext(nc) as tc:
        with (
            tc.tile_pool(name="const", bufs=1) as cpool,
            tc.tile_pool(name="selfb", bufs=1) as spool,
            tc.tile_pool(name="stream", bufs=2) as stpool,
            tc.tile_pool(name="work", bufs=3) as wpool,
            tc.tile_pool(name="node", bufs=3) as npool,
            tc.tile_pool(name="pse", bufs=2, space="PSUM") as pse,
            tc.tile_pool(name="psagg", bufs=2, space="PSUM") as psagg,
            tc.tile_pool(name="psnode", bufs=2, space="PSUM") as psnode,
        ):
            W1_t = cpool.tile([IN_F, HID], f32)
            we1_t = cpool.tile([8, HID], f32)
            bias1_t = cpool.tile([P, HID], f32)
            wcat_t = cpool.tile([HID, P], f32)
            we2_t = cpool.tile([8, P], f32)
            bias2_t = cpool.tile([P, P], f32)
            iota_t = cpool.tile([P, P], f32)
            ident_t = cpool.tile([P, P], f32)
            offs_t = cpool.tile([P, NCH], i32)
            offs2_t = cpool.tile([P, NCH], i32)
            colrel_t = cpool.tile([P, NCH], f32)
            srw_t = cpool.tile([P, NCH], f32)
            dinvcol_t = cpool.tile([P, NBLK], f32)
            rdegc_t = cpool.tile([P, NBLK], f32)
            for t, d in ((W1_t, W1_d), (we1_t, we1_d), (bias1_t, bias1_d),
                         (wcat_t, wcat_d), (we2_t, we2_d), (bias2_t, bias2_d),
                         (iota_t, iota_d), (ident_t, ident_d), (offs_t, offs_d),
                         (offs2_t, offs2_d),
                         (colrel_t, colrel_d), (srw_t, srw_d),
                         (dinvcol_t, dinvcol_d), (rdegc_t, rdegc_d)):
                nc.sync.dma_start(out=t[:], in_=d[:])

            selfbuf1 = [spool.tile([P, HID], f32, name=f"s1_{b}", tag=f"s1_{b}") for b in range(NBLK)]
            selfbuf2 = [spool.tile([P, P], f32, name=f"s2_{b}", tag=f"s2_{b}") for b in range(NBLK)]

            # ---- phase 1b: self term, own shard: relu(x@W1 + b1 + root1) ----
            for b in range(NBLK if "self" in phases else 0):
                xrows = npool.tile([P, HID], f32, tag="xrows")
                lo_s = b * P
                n_s = min(P, SHARD - lo_s)
                nc.sync.dma_start(out=xrows[:n_s, :], in_=xself_d[lo_s:lo_s + n_s, :])
                psx = psnode.tile([P, P], f32, tag="pn")
                nc.tensor.transpose(out=psx[:], in_=xrows[:], identity=ident_t[:])
                xgT = npool.tile([P, P], f32, tag="xgT")
                nc.vector.tensor_copy(out=xgT[:], in_=psx[:])
                psh = psnode.tile([P, HID], f32, tag="pn")
                nc.tensor.matmul(out=psh[:], lhsT=xgT[:], rhs=W1_t[:], start=True, stop=True)
                pre = npool.tile([P, HID], f32, tag="pre1")
                nc.vector.tensor_tensor(out=pre[:], in0=psh[:], in1=bias1_t[:], op=Alu.add)
                nc.scalar.activation(selfbuf1[b][:], pre[:], Relu)

            # ---- edge pass helper ----
            def edge_pass(layer, table, weaug_t, wtab_t, block_done, eoffs_t):
                at_tile = [None]
                cur_at = [-1]
                sup = {}
                pend = []

                def flush(nq):
                    nc.scalar.activation(sup["msg"][:, :nq, :], sup["eps"][:, :nq, :], Relu)
                    for (qq, bb, kk, nkk, agg) in pend:
                        nc.tensor.matmul(
                            out=agg[:], lhsT=sup["S"][:, qq, :],
                            rhs=sup["msg"][:, qq, :],
                            start=(kk == 0), stop=(kk == nkk - 1))
                        if kk == nkk - 1:
                            block_done(bb, agg)
                    pend.clear()

                agg = None
                for cidx, (b, k, nk) in enumerate(sched):
                    q = cidx % SUP
                    if q == 0:
                        sup["g"] = wpool.tile([P, SUP, P], f32, name=f"g{layer}", tag=f"g{layer}")
                        sup["eps"] = pse.tile([P, SUP, P], f32, name="eps", tag="eps")
                        sup["S"] = wpool.tile([P, SUP, P], f32, name=f"S{layer}", tag=f"S{layer}")
                        sup["msg"] = wpool.tile([P, SUP, P], f32, name=f"msg{layer}", tag=f"msg{layer}")
                    if cidx // ATSUP != cur_at[0]:
                        cur_at[0] = cidx // ATSUP
                        lo = cur_at[0] * ATSUP * P
                        n = min(ATSUP * P, NCH * P - lo)
                        at_tile[0] = stpool.tile([8, ATSUP * P], f32, name="at", tag="at")
                        nc.sync.dma_start(out=at_tile[0][:, :n], in_=at_d[:, lo:lo + n])
                    if k == 0:
                        agg = psagg.tile([P, P], f32, tag="agg")
                    if "gather" not in ablate:
                        nc.gpsimd.indirect_dma_start(
                            out=sup["g"][:, q, :], out_offset=None, in_=table[:],
                            in_offset=bass.IndirectOffsetOnAxis(
                                ap=eoffs_t[:, cidx:cidx + 1], axis=0))
                    else:
                        nc.sync.dma_start(out=sup["g"][:, q, :], in_=table[0:P, :])
                    a0 = (cidx - cur_at[0] * ATSUP) * P
                    pst = psnode.tile([P, P], f32, tag="pn", name="pst")
                    nc.tensor.transpose(out=pst[:], in_=sup["g"][:, q, :],
                                        identity=ident_t[:])
                    gT = wpool.tile([P, P], f32, tag=f"gT{layer}", name="gT")
                    nc.vector.tensor_copy(out=gT[:], in_=pst[:])
                    nc.tensor.matmul(out=sup["eps"][:, q, :],
                                     lhsT=at_tile[0][:, a0:a0 + P],
                                     rhs=weaug_t[:], start=True, stop=False)
                    nc.tensor.matmul(out=sup["eps"][:, q, :],
                                     lhsT=gT[:], rhs=wtab_t[:],
                                     start=False, stop=True)
                    if "sgen" not in ablate:
                        nc.vector.tensor_scalar(
                            out=sup["S"][:, q, :], in0=iota_t[:],
                            scalar1=colrel_t[:, cidx:cidx + 1],
                            scalar2=srw_t[:, cidx:cidx + 1],
                            op0=Alu.is_equal, op1=Alu.mult)
                    else:
                        nc.gpsimd.memset(sup["S"][:, q, :], 0.0)
                    pend.append((q, b, k, nk, agg))
                    if q == SUP - 1 or cidx == len(sched) - 1:
                        flush(q + 1)

            # ---- phase 2: layer-1 edge pass; block finals build T2 ----
            def l1_block_done(b, agg):
                u = npool.tile([P, HID], f32, tag="u")
                nc.vector.tensor_scalar(out=u[:], in0=agg[:],
                                        scalar1=dinvcol_t[:, b:b + 1], scalar2=None,
                                        op0=Alu.mult)
                v = npool.tile([P, HID], f32, tag="v")
                nc.vector.tensor_scalar(out=v[:], in0=selfbuf1[b][:],
                                        scalar1=rdegc_t[:, b:b + 1], scalar2=None,
                                        op0=Alu.mult)
                w = npool.tile([P, HID], f32, tag="w")
                nc.vector.tensor_tensor(out=w[:], in0=u[:], in1=v[:], op=Alu.add)
                hb = npool.tile([P, HID], f32, tag="hb")
                nc.scalar.activation(hb[:], w[:], Relu)
                lo = b * P
                n = min(P, SHARD - lo)
                if lo + n <= HALF:
                    nc.sync.dma_start(out=hshardA[lo:lo + n, :], in_=hb[:n, :])
                elif lo >= HALF:
                    nc.sync.dma_start(out=hshardB[lo - HALF:lo - HALF + n, :], in_=hb[:n, :])
                else:
                    nA = HALF - lo
                    nc.sync.dma_start(out=hshardA[lo:HALF, :], in_=hb[:nA, :])
                    nc.sync.dma_start(out=hshardB[0:lo + n - HALF, :], in_=hb[nA:n, :])
                pst = psnode.tile([P, P], f32, tag="pn")
                nc.tensor.transpose(out=pst[:], in_=hb[:], identity=ident_t[:])
                hT = npool.tile([P, P], f32, tag="hT")
                nc.vector.tensor_copy(out=hT[:], in_=pst[:])
                ps2 = psnode.tile([P, P], f32, tag="pn")
                nc.tensor.matmul(out=ps2[:], lhsT=hT[:], rhs=wcat_t[:], start=True, stop=True)
                pre2 = npool.tile([P, P], f32, tag="pre2")
                nc.vector.tensor_tensor(out=pre2[:], in0=ps2[:], in1=bias2_t[:], op=Alu.add)
                nc.scalar.activation(selfbuf2[b][:], pre2[:], Relu)

            if "l1" in phases:
                edge_pass(1, xrows_d, we1_t, W1_t, l1_block_done, offs_t)

            # ---- phase 3: all-gather T2 ----
            if "ag" in phases:
                nc.gpsimd.collective_compute(
                    "AllGather", mybir.AluOpType.bypass,
                    replica_groups=[list(range(N_CORES))],
                    ins=[hshardA[:]], outs=[hfull[0:N_CORES * HALF, :]])
                nc.gpsimd.collective_compute(
                    "AllGather", mybir.AluOpType.bypass,
                    replica_groups=[list(range(N_CORES))],
                    ins=[hshardB[:]], outs=[hfull[N_CORES * HALF:, :]])

            # ---- phase 4: layer-2/3 edge pass ----
            def l2_block_done(b, agg):
                u = npool.tile([P, P], f32, tag="u2")
                nc.vector.tensor_scalar(out=u[:], in0=agg[:],
                                        scalar1=dinvcol_t[:, b:b + 1], scalar2=None,
                                        op0=Alu.mult)
                v = npool.tile([P, P], f32, tag="v2")
                nc.vector.tensor_scalar(out=v[:], in0=selfbuf2[b][:],
                                        scalar1=rdegc_t[:, b:b + 1], scalar2=None,
                                        op0=Alu.mult)
                w = npool.tile([P, P], f32, tag="w2")
                nc.vector.tensor_tensor(out=w[:], in0=u[:], in1=v[:], op=Alu.add)
                lo = b * P
                n = min(P, SHARD - lo)
                nc.sync.dma_start(out=out_d[lo:lo + n, :], in_=w[:n, :])

            if "l2" in phases:
                edge_pass(2, hfull, we2_t, wcat_t, l2_block_done, offs2_t)
            else:
                ztmp = npool.tile([P, P], f32, tag="ztmp")
                nc.vector.memset(ztmp[:], 0.0)
                nc.sync.dma_start(out=out_d[0:P, :], in_=ztmp[:])

    nc.finalize()
    return nc


_CACHE = {}


def kernel(**inputs):
    from concourse.bass_utils import run_bass_kernel_spmd

    per_core, n_chunks, NCH = _host_prep(**inputs)
    key = (tuple(n_chunks), NCH)
    if key not in _CACHE:
        _CACHE[key] = _build_nc(n_chunks, NCH)
    nc = _CACHE[key]
    r = None
    for attempt in range(3):
        try:
            r = run_bass_kernel_spmd(nc, per_core, list(range(N_CORES)))
            break
        except Exception:
            if attempt == 2:
                raise
            import time as _time
            _time.sleep(5.0)
    outs = [r.results[c]["out"] for c in range(N_CORES)]
    full = np.concatenate(outs, axis=0)
    mu = np.ascontiguousarray(full[:, :OUT_F])
    logstd = np.ascontiguousarray(full[:, OUT_F:])
    return (mu, logstd)



# revision 26
# speedup vs baseline: 1.4337x; 1.4337x over previous
"""GCN encoder (3x gcn_conv) on 8 Trainium2 NeuronCores.

Pull-mode graph-parallel layout, gather-free layer 1:
- Edges are grouped by destination core / 128-node local destination block
  (6250 nodes, 49 blocks per core).
- Layer 1: the per-edge source features x[row] are expanded on the HOST into
  a contiguous fp16 stream xTexp [128, NCH*128] (column per edge slot), so
  layer-1 messages need NO device gather: per 128-edge chunk
  eps = at @ We1aug + xTexp_chunk @ W1 accumulates in PSUM, relu -> msg,
  and a one-hot S matmul scatters norm-weighted messages into a per-block
  PSUM accumulator. Block finals produce h and the fused layer-2/3 table
  T2 = h @ [Wmu|Wls] (fp16), written to the core's shard buffer.
- A single fp16 AllGather replicates T2 (rank-padded 6272-row shards).
- Layers 2/3 reuse the SAME edge grouping (same colrel/srw/at); per chunk an
  indirect DMA gathers the 128 T2[row] rows, eps = at @ We2aug + I @ g,
  relu, and the same S scatters into the block accumulator; block finals
  apply dinv[col] and self terms and write the output.
- All matmuls run in fp16 (1 cycle/row vs 4 for fp32) with fp32 PSUM
  accumulation.
"""
import numpy as np

N_NODES = 50000
N_CORES = 8
SHARD = N_NODES // N_CORES          # 6250
P = 128
NBLK = (SHARD + P - 1) // P         # 49 local destination blocks / core
SHARD_PAD = NBLK * P                # 6272
HID = 128
IN_F = 128
OUT_F = 64

GSUP = 16   # chunks per xTexp/gather stream tile
SUP = 4     # chunks per eps/relu batch
ATSUP = 64  # chunks per edge-attr stream tile
TSUP = 8    # xTown stream batch


def _host_prep(x, edge_index, edge_attr,
               W1, b1, We1, be1, root1,
               Wmu, bmu, Wemu, bemu, rootmu,
               Wls, bls, Wels, bels, rootls):
    x = np.asarray(x, np.float32)
    row = np.asarray(edge_index[0], np.int64)
    col = np.asarray(edge_index[1], np.int64)
    ea = np.asarray(edge_attr, np.float32)
    E = row.shape[0]

    deg = (np.bincount(row, minlength=N_NODES) + 1.0).astype(np.float32)
    dinv = deg ** -0.5
    rdeg = (1.0 / deg).astype(np.float32)

    # ---- pull grouping: by destination core, local destination 128-block --
    core_of = col // SHARD
    blk_of = (col - core_of * SHARD) // P
    colrel_v = (col - core_of * SHARD - blk_of * P).astype(np.float32)
    srw_v = dinv[row]

    counts = np.zeros((N_CORES, NBLK), np.int64)
    for c in range(N_CORES):
        m = core_of == c
        counts[c] = np.bincount(blk_of[m], minlength=NBLK)
    n_chunks = np.maximum(1, (counts.max(axis=0) + P - 1) // P).astype(int)
    NCH = int(n_chunks.sum())
    chunk_base = np.concatenate([[0], np.cumsum(n_chunks)])[:-1]

    rowarr = np.zeros((N_CORES, P, NCH), np.int64)
    colrel = np.full((N_CORES, P, NCH), -1.0, np.float32)
    srw = np.zeros((N_CORES, P, NCH), np.float32)
    at = np.zeros((N_CORES, 8, NCH * P), np.float16)

    order = np.lexsort((blk_of, core_of))
    ro, bo, co = row[order], blk_of[order], core_of[order]
    cr, sw, eao = colrel_v[order], srw_v[order], ea[order]
    seg_cnt = np.zeros(N_CORES * NBLK + 1, np.int64)
    np.add.at(seg_cnt, co * NBLK + bo + 1, 1)
    seg_start = np.cumsum(seg_cnt)
    pos_in_seg = np.arange(E) - seg_start[co * NBLK + bo]

    chunk_idx = chunk_base[bo] + pos_in_seg // P
    part_idx = pos_in_seg % P
    rowarr[co, part_idx, chunk_idx] = ro
    colrel[co, part_idx, chunk_idx] = cr
    srw[co, part_idx, chunk_idx] = sw
    flat = chunk_idx * P + part_idx
    for j in range(7):
        at[co, j, flat] = eao[:, j].astype(np.float16)
    at[co, 7, flat] = 1.0

    # layer-2 gather offsets into the rank-padded allgathered T2 table
    offs2 = (rowarr // SHARD) * SHARD_PAD + rowarr % SHARD
    offs2 = offs2.astype(np.int32)

    # host-expanded layer-1 source features: column per edge slot
    xT = np.ascontiguousarray(x.T.astype(np.float16))          # [128, N]
    # xTexp[c][:, cidx*128 + p] = xT[:, rowarr[c, p, cidx]]
    xTexp = [xT[:, rowarr[c].T.ravel()] for c in range(N_CORES)]  # [128, NCH*P]

    # ---- per-core destination-side constants (local blocks, zero-padded) --
    dinvcol = np.zeros((N_CORES, P, NBLK), np.float32)
    rdegc = np.zeros((N_CORES, P, NBLK), np.float32)
    for c in range(N_CORES):
        ids = c * SHARD + np.arange(SHARD)
        b = np.arange(SHARD) // P
        p = np.arange(SHARD) % P
        dinvcol[c, p, b] = dinv[ids]
        rdegc[c, p, b] = rdeg[ids]

    xT_pad = np.zeros((N_CORES, IN_F, SHARD_PAD), np.float16)
    for c in range(N_CORES):
        xT_pad[c, :, :SHARD] = xT[:, c * SHARD:(c + 1) * SHARD]

    W1h = np.asarray(W1, np.float16)
    we1 = np.concatenate([np.asarray(We1, np.float32),
                          (np.asarray(be1) + np.asarray(b1))[None, :]],
                         0).astype(np.float16)
    bias1 = np.tile((np.asarray(b1) + np.asarray(root1))[None, :],
                    (P, 1)).astype(np.float32)
    wcat = np.concatenate([np.asarray(Wmu), np.asarray(Wls)],
                          1).astype(np.float16)
    we2 = np.concatenate([
        np.concatenate([np.asarray(Wemu), np.asarray(Wels)], 1),
        np.concatenate([np.asarray(bemu) + np.asarray(bmu),
                        np.asarray(bels) + np.asarray(bls)])[None, :]],
        0).astype(np.float16)
    bias2 = np.tile(np.concatenate([np.asarray(bmu) + np.asarray(rootmu),
                                    np.asarray(bls) + np.asarray(rootls)])[None, :],
                    (P, 1)).astype(np.float32)
    iota = np.tile(np.arange(P, dtype=np.float16)[None, :], (P, 1))
    ident = np.eye(P, dtype=np.float16)

    shared = dict(W1=W1h, we1=we1, bias1=bias1, wcat=wcat, we2=we2,
                  bias2=bias2, iota=iota, ident=ident)
    per_core = []
    for c in range(N_CORES):
        d = dict(offs2=offs2[c], colrel=colrel[c], srw=srw[c], at=at[c],
                 xTexp=np.ascontiguousarray(xTexp[c]),
                 dinvcol=dinvcol[c], rdegc=rdegc[c],
                 xTown=np.ascontiguousarray(xT_pad[c]))
        d.update(shared)
        per_core.append(d)
    return per_core, tuple(n_chunks), NCH


def _build_nc(n_chunks, NCH, debug=False, ablate=()):
    from concourse import bass, bacc, mybir
    import concourse.tile as tile

    f32 = mybir.dt.float32
    f16 = mybir.dt.float16
    i32 = mybir.dt.int32
    Relu = mybir.ActivationFunctionType.Relu
    Copy = mybir.ActivationFunctionType.Copy
    Alu = mybir.AluOpType
    nc = bacc.Bacc(None, num_devices=N_CORES)

    xTexp_d = nc.declare_dram_parameter("xTexp", [IN_F, NCH * P], f16, isOutput=False)
    xTown_d = nc.declare_dram_parameter("xTown", [IN_F, SHARD_PAD], f16, isOutput=False)
    W1_d = nc.declare_dram_parameter("W1", [IN_F, HID], f16, isOutput=False)
    we1_d = nc.declare_dram_parameter("we1", [8, HID], f16, isOutput=False)
    bias1_d = nc.declare_dram_parameter("bias1", [P, HID], f32, isOutput=False)
    wcat_d = nc.declare_dram_parameter("wcat", [HID, P], f16, isOutput=False)
    we2_d = nc.declare_dram_parameter("we2", [8, P], f16, isOutput=False)
    bias2_d = nc.declare_dram_parameter("bias2", [P, P], f32, isOutput=False)
    iota_d = nc.declare_dram_parameter("iota", [P, P], f16, isOutput=False)
    ident_d = nc.declare_dram_parameter("ident", [P, P], f16, isOutput=False)
    offs2_d = nc.declare_dram_parameter("offs2", [P, NCH], i32, isOutput=False)
    colrel_d = nc.declare_dram_parameter("colrel", [P, NCH], f32, isOutput=False)
    srw_d = nc.declare_dram_parameter("srw", [P, NCH], f32, isOutput=False)
    at_d = nc.declare_dram_parameter("at", [8, NCH * P], f16, isOutput=False)
    dinvcol_d = nc.declare_dram_parameter("dinvcol", [P, NBLK], f32, isOutput=False)
    rdegc_d = nc.declare_dram_parameter("rdegc", [P, NBLK], f32, isOutput=False)
    out_d = nc.declare_dram_parameter("out", [SHARD, P], f32, isOutput=True)

    hshard_d = nc.dram_tensor("hshard", [SHARD_PAD, P], f16)
    t2full_d = nc.dram_tensor("t2full", [N_CORES * SHARD_PAD, P], f16,
                              addr_space="Shared")
    if debug:
        t2_dbg = nc.declare_dram_parameter("t2dbg", [SHARD_PAD, P], f16, isOutput=True)
        gdbg_d = nc.declare_dram_parameter("gdbg", [P, GSUP, P], f16, isOutput=True)

    sched = []
    for b, nk in enumerate(n_chunks):
        for k in range(nk):
            sched.append((b, k, int(nk)))
    nsched = len(sched)

    with tile.TileContext(nc) as tc:
        with (
            tc.tile_pool(name="const", bufs=1) as cpool,
            tc.tile_pool(name="selfb", bufs=1) as spool,
            tc.tile_pool(name="xstream", bufs=2) as xpool,
            tc.tile_pool(name="gat", bufs=2) as gpool,
            tc.tile_pool(name="atstream", bufs=2) as atpool,
            tc.tile_pool(name="work", bufs=3) as wpool,
            tc.tile_pool(name="node", bufs=3) as npool,
            tc.tile_pool(name="pse", bufs=2, space="PSUM") as pse,
            tc.tile_pool(name="psagg", bufs=2, space="PSUM") as psagg,
            tc.tile_pool(name="psnode", bufs=2, space="PSUM") as psnode,
        ):
            W1_t = cpool.tile([IN_F, HID], f16)
            we1_t = cpool.tile([8, HID], f16)
            bias1_t = cpool.tile([P, HID], f32)
            wcat_t = cpool.tile([HID, P], f16)
            we2_t = cpool.tile([8, P], f16)
            bias2_t = cpool.tile([P, P], f32)
            iota_t = cpool.tile([P, P], f16)
            ident_t = cpool.tile([P, P], f16)
            offs2_t = cpool.tile([P, NCH], i32)
            colrel_t = cpool.tile([P, NCH], f32)
            srw_t = cpool.tile([P, NCH], f32)
            dinvcol_t = cpool.tile([P, NBLK], f32)
            rdegc_t = cpool.tile([P, NBLK], f32)
            for t, d in ((W1_t, W1_d), (we1_t, we1_d), (bias1_t, bias1_d),
                         (wcat_t, wcat_d), (we2_t, we2_d), (bias2_t, bias2_d),
                         (iota_t, iota_d), (ident_t, ident_d),
                         (offs2_t, offs2_d), (colrel_t, colrel_d),
                         (srw_t, srw_d),
                         (dinvcol_t, dinvcol_d), (rdegc_t, rdegc_d)):
                nc.sync.dma_start(out=t[:], in_=d[:])

            v1buf = [spool.tile([P, HID], f16, name=f"v1_{b}", tag=f"v1_{b}")
                     for b in range(NBLK)]
            v2buf = [spool.tile([P, P], f16, name=f"v2_{b}", tag=f"v2_{b}")
                     for b in range(NBLK)]

            # ===== phase 1: self terms v1 = relu(x@W1 + b1 + root1)*rdeg ===
            for b0 in range(0, NBLK, TSUP):
                nb = min(TSUP, NBLK - b0)
                xo = xpool.tile([IN_F, TSUP * P], f16, tag="xo")
                nc.sync.dma_start(out=xo[:, :nb * P],
                                  in_=xTown_d[:, b0 * P:(b0 + nb) * P])
                for j in range(nb):
                    b = b0 + j
                    sps = psnode.tile([P, HID], f32, tag="pn")
                    nc.tensor.matmul(out=sps[:],
                                     lhsT=xo[:, j * P:(j + 1) * P],
                                     rhs=W1_t[:], start=True, stop=True)
                    tmp = npool.tile([P, HID], f32, tag="tmp1")
                    nc.vector.tensor_tensor(out=tmp[:], in0=sps[:],
                                            in1=bias1_t[:], op=Alu.add)
                    nc.scalar.activation(v1buf[b][:], tmp[:], Relu,
                                         scale=rdegc_t[:, b:b + 1])

            # ===== edge pass (shared structure for both layers) ============
            def edge_pass(layer, weaug_t, block_done):
                at_tile = [None]
                cur_at = [-1]
                gt = [None]
                sup = {}
                pend = []

                def flush(nq):
                    nc.scalar.activation(sup["msg"][:, :nq, :],
                                         sup["eps"][:, :nq, :], Relu)
                    for (qq, bb, kk, nkk, agg) in pend:
                        nc.tensor.matmul(
                            out=agg[:], lhsT=sup["S"][:, qq, :],
                            rhs=sup["msg"][:, qq, :],
                            start=(kk == 0), stop=(kk == nkk - 1))
                        if kk == nkk - 1:
                            block_done(bb, agg)
                    pend.clear()

                agg = None
                for cidx, (b, k, nk) in enumerate(sched):
                    q = cidx % SUP
                    if q == 0:
                        sup["eps"] = pse.tile([P, SUP, P], f32, name="eps", tag="eps")
                        sup["S"] = wpool.tile([P, SUP, P], f16,
                                              name=f"S{layer}", tag=f"S{layer}")
                        sup["msg"] = wpool.tile([P, SUP, P], f16,
                                                name=f"msg{layer}", tag=f"msg{layer}")
                    gq = cidx % GSUP
                    if gq == 0:
                        gn = min(GSUP, nsched - cidx)
                        if layer == 1 and "l1x" in ablate:
                            if cidx == 0:
                                gt[0] = gpool.tile([IN_F, GSUP * P], f16,
                                                   name="xte", tag="xte")
                                nc.sync.dma_start(out=gt[0][:, :GSUP * P],
                                                  in_=xTexp_d[:, 0:GSUP * P])
                        elif layer == 1:
                            # host-expanded source features, contiguous
                            gt[0] = gpool.tile([IN_F, GSUP * P], f16,
                                               name="xte", tag="xte")
                            nc.sync.dma_start(
                                out=gt[0][:, :gn * P],
                                in_=xTexp_d[:, cidx * P:(cidx + gn) * P])
                        else:
                            gt[0] = gpool.tile([P, GSUP, P], f16,
                                               name="g2", tag="g2")
                    if layer == 2:
                        if "l2gather" in ablate:
                            nc.sync.dma_start(out=gt[0][:, gq, :],
                                              in_=t2full_d[0:P, :])
                        else:
                            nc.gpsimd.indirect_dma_start(
                                out=gt[0][:, gq, :], out_offset=None, in_=t2full_d[:],
                                in_offset=bass.IndirectOffsetOnAxis(
                                    ap=offs2_t[:, cidx:cidx + 1], axis=0))
                        if debug and cidx == 0:
                            nc.sync.dma_start(out=gdbg_d[:], in_=gt[0][:])
                    if cidx // ATSUP != cur_at[0]:
                        cur_at[0] = cidx // ATSUP
                        lo = cur_at[0] * ATSUP * P
                        n = min(ATSUP * P, nsched * P - lo)
                        at_tile[0] = atpool.tile([8, ATSUP * P], f16,
                                                 name="at", tag="at")
                        nc.sync.dma_start(out=at_tile[0][:, :n],
                                          in_=at_d[:, lo:lo + n])
                    if k == 0:
                        agg = psagg.tile([P, P], f32, tag="agg")
                    a0 = (cidx - cur_at[0] * ATSUP) * P
                    nc.tensor.matmul(out=sup["eps"][:, q, :],
                                     lhsT=at_tile[0][:, a0:a0 + P],
                                     rhs=weaug_t[:], start=True, stop=False)
                    if layer == 1 and "l1x" in ablate:
                        nc.tensor.matmul(out=sup["eps"][:, q, :],
                                         lhsT=ident_t[:], rhs=iota_t[:],
                                         start=False, stop=True)
                    elif layer == 1:
                        nc.tensor.matmul(out=sup["eps"][:, q, :],
                                         lhsT=gt[0][:, gq * P:(gq + 1) * P],
                                         rhs=W1_t[:],
                                         start=False, stop=True)
                    else:
                        nc.tensor.matmul(out=sup["eps"][:, q, :],
                                         lhsT=ident_t[:], rhs=gt[0][:, gq, :],
                                         start=False, stop=True)
                    if "sgen" in ablate:
                        nc.vector.memset(sup["S"][:, q, :], 0.0)
                    else:
                        nc.vector.tensor_scalar(
                            out=sup["S"][:, q, :], in0=iota_t[:],
                            scalar1=colrel_t[:, cidx:cidx + 1],
                            scalar2=srw_t[:, cidx:cidx + 1],
                            op0=Alu.is_equal, op1=Alu.mult)
                    pend.append((q, b, k, nk, agg))
                    if q == SUP - 1 or cidx == nsched - 1:
                        flush(q + 1)

            # ===== phase 2: layer-1 pass; block finals build h and T2 ======
            def l1_block_done(b, agg):
                if "l1bd" in ablate:
                    w0 = npool.tile([P, HID], f32, tag="w1")
                    nc.vector.scalar_tensor_tensor(
                        out=w0[:], in0=agg[:], scalar=dinvcol_t[:, b:b + 1],
                        in1=v1buf[b][:], op0=Alu.mult, op1=Alu.add)
                    return
                w = npool.tile([P, HID], f32, tag="w1")
                nc.vector.scalar_tensor_tensor(
                    out=w[:], in0=agg[:], scalar=dinvcol_t[:, b:b + 1],
                    in1=v1buf[b][:], op0=Alu.mult, op1=Alu.add)
                hb = npool.tile([P, HID], f16, tag="hb")
                nc.scalar.activation(hb[:], w[:], Relu)
                pst = psnode.tile([P, P], f16, tag="pnT")
                nc.tensor.transpose(out=pst[:], in_=hb[:], identity=ident_t[:])
                hbT = npool.tile([P, P], f16, tag="hbT")
                nc.scalar.activation(hbT[:], pst[:], Copy)
                t2ps = psnode.tile([P, P], f32, tag="pn")
                nc.tensor.matmul(out=t2ps[:], lhsT=hbT[:], rhs=wcat_t[:],
                                 start=True, stop=True)
                t2sb = npool.tile([P, P], f16, tag="t2sb")
                nc.scalar.activation(t2sb[:], t2ps[:], Copy)
                nc.sync.dma_start(out=hshard_d[b * P:(b + 1) * P, :], in_=t2sb[:])
                if debug:
                    nc.sync.dma_start(out=t2_dbg[b * P:(b + 1) * P, :], in_=t2sb[:])
                t2b = npool.tile([P, P], f32, tag="t2b")
                nc.vector.tensor_tensor(out=t2b[:], in0=t2ps[:],
                                        in1=bias2_t[:], op=Alu.add)
                nc.scalar.activation(v2buf[b][:], t2b[:], Relu,
                                     scale=rdegc_t[:, b:b + 1])

            if "l1" in ablate:
                for b in range(NBLK):
                    zz = npool.tile([P, P], f16, tag="t2sb")
                    nc.vector.memset(zz[:], 0.0)
                    nc.sync.dma_start(out=hshard_d[b * P:(b + 1) * P, :], in_=zz[:])
                    nc.vector.memset(v2buf[b][:], 0.0)
            else:
                edge_pass(1, we1_t, l1_block_done)

            # ===== phase 3: AllGather T2 shards =====
            if "ag" in ablate:
                pass
            else:
                nc.gpsimd.collective_compute(
                "AllGather", Alu.bypass,
                    replica_groups=[list(range(N_CORES))],
                    ins=[hshard_d[:]], outs=[t2full_d[:]])

            # ===== phase 4: layer-2/3 pass; block finals write output ======
            def l2_block_done(b, agg):
                ob = npool.tile([P, P], f32, tag="ob")
                nc.vector.scalar_tensor_tensor(
                    out=ob[:], in0=agg[:], scalar=dinvcol_t[:, b:b + 1],
                    in1=v2buf[b][:], op0=Alu.mult, op1=Alu.add)
                lo = b * P
                n = min(P, SHARD - lo)
                nc.sync.dma_start(out=out_d[lo:lo + n, :], in_=ob[:n, :])

            if "l2" in ablate:
                zo = npool.tile([P, P], f32, tag="ob")
                nc.vector.memset(zo[:], 0.0)
                nc.sync.dma_start(out=out_d[0:P, :], in_=zo[:])
            else:
                edge_pass(2, we2_t, l2_block_done)

    nc.finalize()
    return nc


_CACHE = {}


def kernel(**inputs):
    from concourse.bass_utils import run_bass_kernel_spmd

    per_core, n_chunks, NCH = _host_prep(**inputs)
    key = (n_chunks, NCH)
    if key not in _CACHE:
        _CACHE[key] = _build_nc(n_chunks, NCH)
    nc = _CACHE[key]
    r = None
    for attempt in range(3):
        try:
            r = run_bass_kernel_spmd(nc, per_core, list(range(N_CORES)))
            break
        except Exception:
            if attempt == 2:
                raise
            import time as _time
            _time.sleep(5.0)
    outs = [r.results[c]["out"] for c in range(N_CORES)]
    full = np.concatenate(outs, axis=0)
    mu = np.ascontiguousarray(full[:, :OUT_F])
    logstd = np.ascontiguousarray(full[:, OUT_F:])
    return (mu, logstd)


# revision 29
# speedup vs baseline: 1.5221x; 1.0617x over previous
"""GCN encoder (3x gcn_conv) on 8 Trainium2 NeuronCores.

Pull-mode graph-parallel layout, gather-free layer 1:
- Edges are grouped by destination core / 128-node local destination block
  (6250 nodes, 49 blocks per core).
- Layer 1: the per-edge source features x[row] are expanded on the HOST into
  a contiguous fp16 stream xTexp [128, NCH*128] (column per edge slot), so
  layer-1 messages need NO device gather: per 128-edge chunk
  eps = at @ We1aug + xTexp_chunk @ W1 accumulates in PSUM, relu -> msg,
  and a one-hot S matmul scatters norm-weighted messages into a per-block
  PSUM accumulator. Block finals produce h and the fused layer-2/3 table
  T2 = h @ [Wmu|Wls] (fp16), written to the core's shard buffer.
- A single fp16 AllGather replicates T2 (rank-padded 6272-row shards).
- Layers 2/3 reuse the SAME edge grouping (same colrel/srw/at); per chunk an
  indirect DMA gathers the 128 T2[row] rows, eps = at @ We2aug + I @ g,
  relu, and the same S scatters into the block accumulator; block finals
  apply dinv[col] and self terms and write the output.
- All matmuls run in fp16 (1 cycle/row vs 4 for fp32) with fp32 PSUM
  accumulation.
"""
import numpy as np

N_NODES = 50000
N_CORES = 8
SHARD = N_NODES // N_CORES          # 6250
P = 128
NBLK = (SHARD + P - 1) // P         # 49 local destination blocks / core
SHARD_PAD = NBLK * P                # 6272
HID = 128
IN_F = 128
OUT_F = 64

AG_BOUNDS = [0, 12, 24, 36, 49]  # AllGather piece boundaries (local blocks)
GSUP = 16   # chunks per xTexp/gather stream tile
SUP = 4     # chunks per eps/relu batch
ATSUP = 64  # chunks per edge-attr stream tile
TSUP = 8    # xTown stream batch


def _host_prep(x, edge_index, edge_attr,
               W1, b1, We1, be1, root1,
               Wmu, bmu, Wemu, bemu, rootmu,
               Wls, bls, Wels, bels, rootls):
    x = np.asarray(x, np.float32)
    row = np.asarray(edge_index[0], np.int64)
    col = np.asarray(edge_index[1], np.int64)
    ea = np.asarray(edge_attr, np.float32)
    E = row.shape[0]

    deg = (np.bincount(row, minlength=N_NODES) + 1.0).astype(np.float32)
    dinv = deg ** -0.5
    rdeg = (1.0 / deg).astype(np.float32)

    # ---- pull grouping: by destination core, local destination 128-block --
    core_of = col // SHARD
    blk_of = (col - core_of * SHARD) // P
    colrel_v = (col - core_of * SHARD - blk_of * P).astype(np.float32)
    srw_v = dinv[row]

    counts = np.zeros((N_CORES, NBLK), np.int64)
    for c in range(N_CORES):
        m = core_of == c
        counts[c] = np.bincount(blk_of[m], minlength=NBLK)
    n_chunks = np.maximum(1, (counts.max(axis=0) + P - 1) // P).astype(int)
    NCH = int(n_chunks.sum())
    chunk_base = np.concatenate([[0], np.cumsum(n_chunks)])[:-1]

    rowarr = np.zeros((N_CORES, P, NCH), np.int64)
    colrel = np.full((N_CORES, P, NCH), -1.0, np.float32)
    srw = np.zeros((N_CORES, P, NCH), np.float32)
    at = np.zeros((N_CORES, 8, NCH * P), np.float16)

    order = np.lexsort((blk_of, core_of))
    ro, bo, co = row[order], blk_of[order], core_of[order]
    cr, sw, eao = colrel_v[order], srw_v[order], ea[order]
    seg_cnt = np.zeros(N_CORES * NBLK + 1, np.int64)
    np.add.at(seg_cnt, co * NBLK + bo + 1, 1)
    seg_start = np.cumsum(seg_cnt)
    pos_in_seg = np.arange(E) - seg_start[co * NBLK + bo]

    chunk_idx = chunk_base[bo] + pos_in_seg // P
    part_idx = pos_in_seg % P
    rowarr[co, part_idx, chunk_idx] = ro
    colrel[co, part_idx, chunk_idx] = cr
    srw[co, part_idx, chunk_idx] = sw
    flat = chunk_idx * P + part_idx
    for j in range(7):
        at[co, j, flat] = eao[:, j].astype(np.float16)
    at[co, 7, flat] = 1.0

    # layer-2 gather offsets into the piece-major allgathered T2 table:
    # t2full rows are [piece][rank][rows-in-piece]; pieces split the local
    # blocks at AG_BOUNDS
    bounds = np.array(AG_BOUNDS, np.int64) * P          # local-row bounds
    rk = rowarr // SHARD
    rl = rowarr % SHARD
    pc = np.searchsorted(bounds, rl, side="right") - 1  # piece index
    rows_i = (bounds[1:] - bounds[:-1])                 # rows per rank per piece
    base = np.concatenate([[0], np.cumsum(rows_i * N_CORES)])[:-1]
    offs2 = (base[pc] + rk * rows_i[pc] + (rl - bounds[pc])).astype(np.int32)

    # host-expanded layer-1 source features: column per edge slot
    xT = np.ascontiguousarray(x.T.astype(np.float16))          # [128, N]
    # xTexp[c][:, cidx*128 + p] = xT[:, rowarr[c, p, cidx]]
    xTexp = [xT[:, rowarr[c].T.ravel()] for c in range(N_CORES)]  # [128, NCH*P]

    # ---- per-core destination-side constants (local blocks, zero-padded) --
    dinvcol = np.zeros((N_CORES, P, NBLK), np.float32)
    rdegc = np.zeros((N_CORES, P, NBLK), np.float32)
    for c in range(N_CORES):
        ids = c * SHARD + np.arange(SHARD)
        b = np.arange(SHARD) // P
        p = np.arange(SHARD) % P
        dinvcol[c, p, b] = dinv[ids]
        rdegc[c, p, b] = rdeg[ids]

    xT_pad = np.zeros((N_CORES, IN_F, SHARD_PAD), np.float16)
    for c in range(N_CORES):
        xT_pad[c, :, :SHARD] = xT[:, c * SHARD:(c + 1) * SHARD]

    W1h = np.asarray(W1, np.float16)
    we1 = np.concatenate([np.asarray(We1, np.float32),
                          (np.asarray(be1) + np.asarray(b1))[None, :]],
                         0).astype(np.float16)
    bias1 = np.tile((np.asarray(b1) + np.asarray(root1))[None, :],
                    (P, 1)).astype(np.float32)
    wcat = np.concatenate([np.asarray(Wmu), np.asarray(Wls)],
                          1).astype(np.float16)
    we2 = np.concatenate([
        np.concatenate([np.asarray(Wemu), np.asarray(Wels)], 1),
        np.concatenate([np.asarray(bemu) + np.asarray(bmu),
                        np.asarray(bels) + np.asarray(bls)])[None, :]],
        0).astype(np.float16)
    bias2 = np.tile(np.concatenate([np.asarray(bmu) + np.asarray(rootmu),
                                    np.asarray(bls) + np.asarray(rootls)])[None, :],
                    (P, 1)).astype(np.float32)
    iota = np.tile(np.arange(P, dtype=np.float16)[None, :], (P, 1))
    ident = np.eye(P, dtype=np.float16)

    shared = dict(W1=W1h, we1=we1, bias1=bias1, wcat=wcat, we2=we2,
                  bias2=bias2, iota=iota, ident=ident)
    per_core = []
    for c in range(N_CORES):
        d = dict(offs2=offs2[c], colrel=colrel[c], srw=srw[c], at=at[c],
                 xTexp=np.ascontiguousarray(xTexp[c]),
                 dinvcol=dinvcol[c], rdegc=rdegc[c],
                 xTown=np.ascontiguousarray(xT_pad[c]))
        d.update(shared)
        per_core.append(d)
    return per_core, tuple(n_chunks), NCH


def _build_nc(n_chunks, NCH, debug=False, ablate=()):
    from concourse import bass, bacc, mybir
    import concourse.tile as tile

    f32 = mybir.dt.float32
    f16 = mybir.dt.float16
    i32 = mybir.dt.int32
    Relu = mybir.ActivationFunctionType.Relu
    Copy = mybir.ActivationFunctionType.Copy
    Alu = mybir.AluOpType
    nc = bacc.Bacc(None, num_devices=N_CORES)

    xTexp_d = nc.declare_dram_parameter("xTexp", [IN_F, NCH * P], f16, isOutput=False)
    xTown_d = nc.declare_dram_parameter("xTown", [IN_F, SHARD_PAD], f16, isOutput=False)
    W1_d = nc.declare_dram_parameter("W1", [IN_F, HID], f16, isOutput=False)
    we1_d = nc.declare_dram_parameter("we1", [8, HID], f16, isOutput=False)
    bias1_d = nc.declare_dram_parameter("bias1", [P, HID], f32, isOutput=False)
    wcat_d = nc.declare_dram_parameter("wcat", [HID, P], f16, isOutput=False)
    we2_d = nc.declare_dram_parameter("we2", [8, P], f16, isOutput=False)
    bias2_d = nc.declare_dram_parameter("bias2", [P, P], f32, isOutput=False)
    iota_d = nc.declare_dram_parameter("iota", [P, P], f16, isOutput=False)
    ident_d = nc.declare_dram_parameter("ident", [P, P], f16, isOutput=False)
    offs2_d = nc.declare_dram_parameter("offs2", [P, NCH], i32, isOutput=False)
    colrel_d = nc.declare_dram_parameter("colrel", [P, NCH], f32, isOutput=False)
    srw_d = nc.declare_dram_parameter("srw", [P, NCH], f32, isOutput=False)
    at_d = nc.declare_dram_parameter("at", [8, NCH * P], f16, isOutput=False)
    dinvcol_d = nc.declare_dram_parameter("dinvcol", [P, NBLK], f32, isOutput=False)
    rdegc_d = nc.declare_dram_parameter("rdegc", [P, NBLK], f32, isOutput=False)
    out_d = nc.declare_dram_parameter("out", [SHARD, P], f32, isOutput=True)

    hshard_ps = [nc.dram_tensor(f"hshard{i}",
                                [(AG_BOUNDS[i + 1] - AG_BOUNDS[i]) * P, P], f16)
                 for i in range(4)]
    t2full_d = nc.dram_tensor("t2full", [N_CORES * SHARD_PAD, P], f16,
                              addr_space="Shared")
    if debug:
        t2_dbg = nc.declare_dram_parameter("t2dbg", [SHARD_PAD, P], f16, isOutput=True)
        gdbg_d = nc.declare_dram_parameter("gdbg", [P, GSUP, P], f16, isOutput=True)

    sched = []
    for b, nk in enumerate(n_chunks):
        for k in range(nk):
            sched.append((b, k, int(nk)))
    nsched = len(sched)

    with tile.TileContext(nc) as tc:
        with (
            tc.tile_pool(name="const", bufs=1) as cpool,
            tc.tile_pool(name="selfb", bufs=1) as spool,
            tc.tile_pool(name="xstream", bufs=2) as xpool,
            tc.tile_pool(name="gat", bufs=2) as gpool,
            tc.tile_pool(name="atstream", bufs=2) as atpool,
            tc.tile_pool(name="work", bufs=3) as wpool,
            tc.tile_pool(name="node", bufs=3) as npool,
            tc.tile_pool(name="pse", bufs=2, space="PSUM") as pse,
            tc.tile_pool(name="psagg", bufs=2, space="PSUM") as psagg,
            tc.tile_pool(name="psnode", bufs=2, space="PSUM") as psnode,
        ):
            W1_t = cpool.tile([IN_F, HID], f16)
            we1_t = cpool.tile([8, HID], f16)
            bias1_t = cpool.tile([P, HID], f32)
            wcat_t = cpool.tile([HID, P], f16)
            we2_t = cpool.tile([8, P], f16)
            bias2_t = cpool.tile([P, P], f32)
            iota_t = cpool.tile([P, P], f16)
            ident_t = cpool.tile([P, P], f16)
            offs2_t = cpool.tile([P, NCH], i32)
            colrel_t = cpool.tile([P, NCH], f32)
            srw_t = cpool.tile([P, NCH], f32)
            dinvcol_t = cpool.tile([P, NBLK], f32)
            rdegc_t = cpool.tile([P, NBLK], f32)
            for t, d in ((W1_t, W1_d), (we1_t, we1_d), (bias1_t, bias1_d),
                         (wcat_t, wcat_d), (we2_t, we2_d), (bias2_t, bias2_d),
                         (iota_t, iota_d), (ident_t, ident_d),
                         (offs2_t, offs2_d), (colrel_t, colrel_d),
                         (srw_t, srw_d),
                         (dinvcol_t, dinvcol_d), (rdegc_t, rdegc_d)):
                nc.sync.dma_start(out=t[:], in_=d[:])

            v1buf = [spool.tile([P, HID], f16, name=f"v1_{b}", tag=f"v1_{b}")
                     for b in range(NBLK)]
            v2buf = [spool.tile([P, P], f16, name=f"v2_{b}", tag=f"v2_{b}")
                     for b in range(NBLK)]

            # ===== phase 1: self terms v1 = relu(x@W1 + b1 + root1)*rdeg ===
            for b0 in range(0, NBLK, TSUP):
                nb = min(TSUP, NBLK - b0)
                xo = xpool.tile([IN_F, TSUP * P], f16, tag="xo")
                nc.sync.dma_start(out=xo[:, :nb * P],
                                  in_=xTown_d[:, b0 * P:(b0 + nb) * P])
                for j in range(nb):
                    b = b0 + j
                    sps = psnode.tile([P, HID], f32, tag="pn")
                    nc.tensor.matmul(out=sps[:],
                                     lhsT=xo[:, j * P:(j + 1) * P],
                                     rhs=W1_t[:], start=True, stop=True)
                    tmp = npool.tile([P, HID], f32, tag="tmp1")
                    nc.vector.tensor_tensor(out=tmp[:], in0=sps[:],
                                            in1=bias1_t[:], op=Alu.add)
                    nc.scalar.activation(v1buf[b][:], tmp[:], Relu,
                                         scale=rdegc_t[:, b:b + 1])

            # ===== edge pass (shared structure for both layers) ============
            def edge_pass(layer, weaug_t, block_done):
                at_tile = [None]
                cur_at = [-1]
                gt = [None]
                sup = {}
                pend = []

                def flush(nq):
                    nc.scalar.activation(sup["msg"][:, :nq, :],
                                         sup["eps"][:, :nq, :], Relu)
                    for (qq, bb, kk, nkk, agg) in pend:
                        nc.tensor.matmul(
                            out=agg[:], lhsT=sup["S"][:, qq, :],
                            rhs=sup["msg"][:, qq, :],
                            start=(kk == 0), stop=(kk == nkk - 1))
                        if kk == nkk - 1:
                            block_done(bb, agg)
                    pend.clear()

                agg = None
                for cidx, (b, k, nk) in enumerate(sched):
                    q = cidx % SUP
                    if q == 0:
                        sup["eps"] = pse.tile([P, SUP, P], f32, name="eps", tag="eps")
                        sup["S"] = wpool.tile([P, SUP, P], f16,
                                              name=f"S{layer}", tag=f"S{layer}")
                        sup["msg"] = wpool.tile([P, SUP, P], f16,
                                                name=f"msg{layer}", tag=f"msg{layer}")
                    gq = cidx % GSUP
                    if gq == 0:
                        gn = min(GSUP, nsched - cidx)
                        if layer == 1 and "l1x" in ablate:
                            if cidx == 0:
                                gt[0] = gpool.tile([IN_F, GSUP * P], f16,
                                                   name="xte", tag="xte")
                                nc.sync.dma_start(out=gt[0][:, :GSUP * P],
                                                  in_=xTexp_d[:, 0:GSUP * P])
                        elif layer == 1:
                            # host-expanded source features, contiguous
                            gt[0] = gpool.tile([IN_F, GSUP * P], f16,
                                               name="xte", tag="xte")
                            nc.sync.dma_start(
                                out=gt[0][:, :gn * P],
                                in_=xTexp_d[:, cidx * P:(cidx + gn) * P])
                        else:
                            gt[0] = gpool.tile([P, GSUP, P], f16,
                                               name="g2", tag="g2")
                    if layer == 2:
                        if "l2gather" in ablate:
                            nc.sync.dma_start(out=gt[0][:, gq, :],
                                              in_=t2full_d[0:P, :])
                        else:
                            nc.gpsimd.indirect_dma_start(
                                out=gt[0][:, gq, :], out_offset=None, in_=t2full_d[:],
                                in_offset=bass.IndirectOffsetOnAxis(
                                    ap=offs2_t[:, cidx:cidx + 1], axis=0))
                        if debug and cidx == 0:
                            nc.sync.dma_start(out=gdbg_d[:], in_=gt[0][:])
                    if cidx // ATSUP != cur_at[0]:
                        cur_at[0] = cidx // ATSUP
                        lo = cur_at[0] * ATSUP * P
                        n = min(ATSUP * P, nsched * P - lo)
                        at_tile[0] = atpool.tile([8, ATSUP * P], f16,
                                                 name="at", tag="at")
                        nc.sync.dma_start(out=at_tile[0][:, :n],
                                          in_=at_d[:, lo:lo + n])
                    if k == 0:
                        agg = psagg.tile([P, P], f32, tag="agg")
                    a0 = (cidx - cur_at[0] * ATSUP) * P
                    nc.tensor.matmul(out=sup["eps"][:, q, :],
                                     lhsT=at_tile[0][:, a0:a0 + P],
                                     rhs=weaug_t[:], start=True, stop=False)
                    if layer == 1 and "l1x" in ablate:
                        nc.tensor.matmul(out=sup["eps"][:, q, :],
                                         lhsT=ident_t[:], rhs=iota_t[:],
                                         start=False, stop=True)
                    elif layer == 1:
                        nc.tensor.matmul(out=sup["eps"][:, q, :],
                                         lhsT=gt[0][:, gq * P:(gq + 1) * P],
                                         rhs=W1_t[:],
                                         start=False, stop=True)
                    else:
                        nc.tensor.matmul(out=sup["eps"][:, q, :],
                                         lhsT=ident_t[:], rhs=gt[0][:, gq, :],
                                         start=False, stop=True)
                    if "sgen" in ablate:
                        nc.vector.memset(sup["S"][:, q, :], 0.0)
                    else:
                        nc.vector.tensor_scalar(
                            out=sup["S"][:, q, :], in0=iota_t[:],
                            scalar1=colrel_t[:, cidx:cidx + 1],
                            scalar2=srw_t[:, cidx:cidx + 1],
                            op0=Alu.is_equal, op1=Alu.mult)
                    pend.append((q, b, k, nk, agg))
                    if q == SUP - 1 or cidx == nsched - 1:
                        flush(q + 1)

            # ===== phase 2: layer-1 pass; block finals build h and T2 ======
            def l1_block_done(b, agg):
                if "l1bd" in ablate:
                    w0 = npool.tile([P, HID], f32, tag="w1")
                    nc.vector.scalar_tensor_tensor(
                        out=w0[:], in0=agg[:], scalar=dinvcol_t[:, b:b + 1],
                        in1=v1buf[b][:], op0=Alu.mult, op1=Alu.add)
                    return
                w = npool.tile([P, HID], f32, tag="w1")
                nc.vector.scalar_tensor_tensor(
                    out=w[:], in0=agg[:], scalar=dinvcol_t[:, b:b + 1],
                    in1=v1buf[b][:], op0=Alu.mult, op1=Alu.add)
                hb = npool.tile([P, HID], f16, tag="hb")
                nc.scalar.activation(hb[:], w[:], Relu)
                pst = psnode.tile([P, P], f16, tag="pnT")
                nc.tensor.transpose(out=pst[:], in_=hb[:], identity=ident_t[:])
                hbT = npool.tile([P, P], f16, tag="hbT")
                nc.scalar.activation(hbT[:], pst[:], Copy)
                t2ps = psnode.tile([P, P], f32, tag="pn")
                nc.tensor.matmul(out=t2ps[:], lhsT=hbT[:], rhs=wcat_t[:],
                                 start=True, stop=True)
                t2sb = npool.tile([P, P], f16, tag="t2sb")
                nc.scalar.activation(t2sb[:], t2ps[:], Copy)
                pi = next(i for i in range(4)
                          if AG_BOUNDS[i] <= b < AG_BOUNDS[i + 1])
                b0 = b - AG_BOUNDS[pi]
                nc.sync.dma_start(out=hshard_ps[pi][b0 * P:(b0 + 1) * P, :],
                                  in_=t2sb[:])
                if debug:
                    nc.sync.dma_start(out=t2_dbg[b * P:(b + 1) * P, :], in_=t2sb[:])
                t2b = npool.tile([P, P], f32, tag="t2b")
                nc.vector.tensor_tensor(out=t2b[:], in0=t2ps[:],
                                        in1=bias2_t[:], op=Alu.add)
                nc.scalar.activation(v2buf[b][:], t2b[:], Relu,
                                     scale=rdegc_t[:, b:b + 1])

            if "l1" in ablate:
                for b in range(NBLK):
                    zz = npool.tile([P, P], f16, tag="t2sb")
                    nc.vector.memset(zz[:], 0.0)
                    pi = next(i for i in range(4)
                              if AG_BOUNDS[i] <= b < AG_BOUNDS[i + 1])
                    nc.sync.dma_start(
                        out=hshard_ps[pi][(b - AG_BOUNDS[pi]) * P:
                                          (b - AG_BOUNDS[pi] + 1) * P, :],
                        in_=zz[:])
                    nc.vector.memset(v2buf[b][:], 0.0)
            else:
                edge_pass(1, we1_t, l1_block_done)

            # ===== phase 3: piecewise AllGather of T2 shards =====
            if "ag" not in ablate:
                base = 0
                for i in range(4):
                    rows = (AG_BOUNDS[i + 1] - AG_BOUNDS[i]) * P
                    nc.gpsimd.collective_compute(
                        "AllGather", Alu.bypass,
                        replica_groups=[list(range(N_CORES))],
                        ins=[hshard_ps[i][:]],
                        outs=[t2full_d[base:base + N_CORES * rows, :]])
                    base += N_CORES * rows

            # ===== phase 4: layer-2/3 pass; block finals write output ======
            def l2_block_done(b, agg):
                ob = npool.tile([P, P], f32, tag="ob")
                nc.vector.scalar_tensor_tensor(
                    out=ob[:], in0=agg[:], scalar=dinvcol_t[:, b:b + 1],
                    in1=v2buf[b][:], op0=Alu.mult, op1=Alu.add)
                lo = b * P
                n = min(P, SHARD - lo)
                nc.sync.dma_start(out=out_d[lo:lo + n, :], in_=ob[:n, :])

            if "l2" in ablate:
                zo = npool.tile([P, P], f32, tag="ob")
                nc.vector.memset(zo[:], 0.0)
                nc.sync.dma_start(out=out_d[0:P, :], in_=zo[:])
            else:
                edge_pass(2, we2_t, l2_block_done)

    nc.finalize()
    return nc


_CACHE = {}


def kernel(**inputs):
    from concourse.bass_utils import run_bass_kernel_spmd

    per_core, n_chunks, NCH = _host_prep(**inputs)
    key = (n_chunks, NCH)
    if key not in _CACHE:
        _CACHE[key] = _build_nc(n_chunks, NCH)
    nc = _CACHE[key]
    r = None
    for attempt in range(3):
        try:
            r = run_bass_kernel_spmd(nc, per_core, list(range(N_CORES)))
            break
        except Exception:
            if attempt == 2:
                raise
            import time as _time
            _time.sleep(5.0)
    outs = [r.results[c]["out"] for c in range(N_CORES)]
    full = np.concatenate(outs, axis=0)
    mu = np.ascontiguousarray(full[:, :OUT_F])
    logstd = np.ascontiguousarray(full[:, OUT_F:])
    return (mu, logstd)
